# revision 1
# baseline (speedup 1.0000x reference)
"""AMPBlock0 (BigVGAN) Trainium2 kernel: B=8 data-parallel over 8 NeuronCores.

Per core: x (512, 8192) f32 -> out (512, 8188) f32
  a1 = down1(snake1(up1(x)))       # polyphase up x2, SnakeBeta, stride-2 lowpass
  c1 = conv1d_3tap(a1) + b1
  a2 = down2(snake2(up2(c1)))
  out = conv1d_3tap(a2) + b2 + x[:, :8188]

Layout: channels on partitions (4 blocks x 128), time on free axis.
Time tiled (L=1024) with halos. bf16 storage/matmul, f32 PSUM.
Engine split (v2): PE = dense convs + 12-tap down convs (diag matmuls);
DVE = up-conv middle taps (STT chains); ACT = first up-tap (Identity w/
per-channel scale+bias), cos via Sin(scale*acc + bias'), psum evicts;
Pool = snake combine (mul-bcast + add), last up-tap, residual add.
"""

import sys

if "/opt/trn_rl_repo" not in sys.path:
    sys.path.insert(0, "/opt/trn_rl_repo")

import numpy as np
import ml_dtypes

import concourse.bacc as bacc
import concourse.mybir as mybir
import concourse.tile as tile
from concourse.bass_utils import run_bass_kernel_spmd

BF16 = mybir.dt.bfloat16
F32 = mybir.dt.float32
AF = mybir.ActivationFunctionType
ALU = mybir.AluOpType

T = 8192
C = 512
NB = 4
L = 1024
NT = T // L
PAD = 16
TOUT = T - 4
CHUNK = 512

# sc columns (128, NB, 64) f32; stage offset S2=28
# 0-5 we, 6-11 wo, 12-17 d_o, 18-23 d_e, 24 scaleA, 25 biasS, 26 ninv2b, 27 inv2b
# 56 bias1(cout), 57 bias2(cout)
S2 = 28

LAST_EXEC_NS = None
LAST_PROFILE = None


def _chunks(width):
    out, c0 = [], 0
    while c0 < width:
        out.append((c0, min(CHUNK, width - c0)))
        c0 += CHUNK
    return out


def build_graph():
    nc = bacc.Bacc()
    xp_d = nc.declare_dram_parameter("xp", [128, NB, T + 2 * PAD], BF16, isOutput=False)
    w1t_d = nc.declare_dram_parameter("w1t", [128, 3, NB, NB, 128], BF16, isOutput=False)
    w2t_d = nc.declare_dram_parameter("w2t", [128, 3, NB, NB, 128], BF16, isOutput=False)
    sc_d = nc.declare_dram_parameter("sc", [128, NB, 64], F32, isOutput=False)
    diag_d = nc.declare_dram_parameter("diag", [128, 2 * NB * 12, 128], BF16, isOutput=False)
    out_d = nc.declare_dram_parameter("out", [128, NB, TOUT], BF16, isOutput=True)

    with tile.TileContext(nc) as tc:
        with (
            tc.tile_pool(name="const", bufs=1) as constp,
            tc.tile_pool(name="xt", bufs=2) as xtp,
            tc.tile_pool(name="acc_e", bufs=2) as accep,
            tc.tile_pool(name="acc_o", bufs=2) as accop,
            tc.tile_pool(name="cos", bufs=2) as cosp,
            tc.tile_pool(name="tmp", bufs=4) as tmpp,
            tc.tile_pool(name="sE", bufs=2) as sEp,
            tc.tile_pool(name="sO", bufs=2) as sOp,
            tc.tile_pool(name="amid", bufs=2) as amidp,
            tc.tile_pool(name="c1", bufs=2) as c1p,
            tc.tile_pool(name="outt", bufs=2) as outp,
            tc.tile_pool(name="dg", bufs=2) as dgp,
            tc.tile_pool(name="dps", bufs=3, space="PSUM") as dpsp,
            tc.tile_pool(name="wps", bufs=3, space="PSUM") as wpsp,
        ):
            accpools = {"acc_e": accep, "acc_o": accop}

            w1t = constp.tile([128, 3, NB, NB, 128], BF16)
            nc.sync.dma_start(w1t[:], w1t_d[:])
            w2t = constp.tile([128, 3, NB, NB, 128], BF16)
            nc.sync.dma_start(w2t[:], w2t_d[:])
            sc_t = constp.tile([128, NB, 64], F32)
            nc.sync.dma_start(sc_t[:], sc_d[:])

            def upconv_snake(b, src_tile, width, off, E, O):
                """One block's up-convs (both phases) + snake into E/O.
                off = stage scalar-column offset (0 or S2)."""
                for phase, dst in ((0, E), (1, O)):
                    wb = off + 6 * phase
                    tag = "acc_e" if phase == 0 else "acc_o"
                    accp = accpools[tag]
                    # tap 0 on ACT: acc = we0*x + inv2b
                    acc = accp.tile([128, NB, width], BF16, tag=tag)
                    nc.scalar.activation(
                        acc[:, b, :], src_tile[:, b, 0:width], AF.Identity,
                        bias=sc_t[:, b, off + 27:off + 28],
                        scale=sc_t[:, b, wb:wb + 1],
                    )
                    cur = acc
                    # taps 1-4 on DVE (STT)
                    for k in range(1, 5):
                        nxt = accp.tile([128, NB, width], BF16, tag=tag)
                        nc.vector.scalar_tensor_tensor(
                            nxt[:, b, :], src_tile[:, b, k:k + width],
                            sc_t[:, b, wb + k:wb + k + 1], cur[:, b, :],
                            ALU.mult, ALU.add,
                        )
                        cur = nxt
                    # tap 5 on Pool: mul(bcast) + add
                    pt = tmpp.tile([128, width], BF16, tag="tmp")
                    nc.gpsimd.tensor_mul(
                        pt[:, :], src_tile[:, b, 5:5 + width],
                        sc_t[:, b, wb + 5:wb + 6].broadcast_to([128, width]),
                    )
                    fin = accp.tile([128, NB, width], BF16, tag=tag)
                    nc.gpsimd.tensor_add(fin[:, b, :], pt[:, :], cur[:, b, :])
                    # cos on ACT
                    cost = cosp.tile([128, NB, width], BF16, tag="cos")
                    nc.scalar.activation(
                        cost[:, b, :], fin[:, b, :], AF.Sin,
                        bias=sc_t[:, b, off + 25:off + 26],
                        scale=sc_t[:, b, off + 24:off + 25],
                    )
                    # snake combine on Pool: dst = cos*(-inv2b) + acc
                    st = tmpp.tile([128, width], BF16, tag="tmp")
                    nc.gpsimd.tensor_mul(
                        st[:, :], cost[:, b, :],
                        sc_t[:, b, off + 26:off + 27].broadcast_to([128, width]),
                    )
                    nc.gpsimd.tensor_add(dst[:, b, :], st[:, :], fin[:, b, :])

            def downconv(b, E, O, width, dgt, dst):
                """12-tap two-phase down conv on PE -> dst (via ACT evict)."""
                for c0, n in _chunks(width):
                    ps = wpsp.tile([128, CHUNK], F32, tag="wps")
                    for r in range(6):
                        nc.tensor.matmul(
                            ps[:, :n], dgt[:, b * 12 + r, :],
                            O[:, b, c0 + r:c0 + r + n],
                            start=(r == 0), stop=False,
                        )
                    for r in range(6):
                        nc.tensor.matmul(
                            ps[:, :n], dgt[:, b * 12 + 6 + r, :],
                            E[:, b, c0 + r + 1:c0 + r + 1 + n],
                            start=False, stop=(r == 5),
                        )
                    nc.scalar.copy(dst[:, b, c0:c0 + n], ps[:, :n])

            for i in range(NT):
                t0 = i * L
                first, last = i == 0, i == NT - 1
                W1, s1 = L + 21, t0 - 8
                W2, s2 = L + 15, t0 - 6
                W3, s3 = L + 13, t0 - 5
                W4, s4 = L + 8, t0 - 3
                W5, s5 = L + 2, t0 - 1
                Wx, sx = L + 26, t0 - 10

                xt = xtp.tile([128, NB, Wx], BF16)
                nc.sync.dma_start(xt[:], xp_d[:, :, sx + PAD:sx + PAD + Wx])

                # stage 1 up + snake
                E1 = sEp.tile([128, NB, W1], BF16, tag="sE")
                O1 = sOp.tile([128, NB, W1], BF16, tag="sO")
                for b in range(NB):
                    upconv_snake(b, xt, W1, 0, E1, O1)
                if first:
                    nc.gpsimd.memset(E1[:, :, 0:1 - s1], 0.0)
                    nc.gpsimd.memset(O1[:, :, 0:1 - s1], 0.0)
                if last:
                    z = (T - 1) - s1
                    nc.gpsimd.memset(E1[:, :, z:W1], 0.0)
                    nc.gpsimd.memset(O1[:, :, z:W1], 0.0)

                # down1 -> a1
                dg1 = dgp.tile([128, NB * 12, 128], BF16, tag="dg")
                nc.sync.dma_start(dg1[:], diag_d[:, 0:NB * 12, :])
                a1 = amidp.tile([128, NB, W2], BF16, tag="amid")
                for b in range(NB):
                    downconv(b, E1, O1, W2, dg1, a1)
                if first:
                    nc.gpsimd.memset(a1[:, :, 0:0 - s2], 0.0)
                if last:
                    z = (T - 2) - s2
                    nc.gpsimd.memset(a1[:, :, z:W2], 0.0)

                # conv1 -> c1 (+bias1)
                c1 = c1p.tile([128, NB, W3], BF16, tag="c1")
                for o in range(NB):
                    for c0, n in _chunks(W3):
                        ps = dpsp.tile([128, CHUNK], F32, tag="dps")
                        for idx, (ib, k) in enumerate(
                            (ib, k) for ib in range(NB) for k in range(3)
                        ):
                            nc.tensor.matmul(
                                ps[:, :n], w1t[:, k, ib, o, :],
                                a1[:, ib, c0 + k:c0 + k + n],
                                start=(idx == 0), stop=(idx == 11),
                            )
                        nc.scalar.activation(
                            c1[:, o, c0:c0 + n], ps[:, :n], AF.Identity,
                            bias=sc_t[:, o, 56:57], scale=1.0,
                        )
                if first:
                    nc.gpsimd.memset(c1[:, :, 0:0 - s3], 0.0)
                if last:
                    z = (T - 2) - s3
                    nc.gpsimd.memset(c1[:, :, z:W3], 0.0)

                # stage 2 up + snake
                E2 = sEp.tile([128, NB, W4], BF16, tag="sE")
                O2 = sOp.tile([128, NB, W4], BF16, tag="sO")
                for b in range(NB):
                    upconv_snake(b, c1, W4, S2, E2, O2)
                if first:
                    nc.gpsimd.memset(E2[:, :, 0:1 - s4], 0.0)
                    nc.gpsimd.memset(O2[:, :, 0:1 - s4], 0.0)
                if last:
                    z = (T - 3) - s4
                    nc.gpsimd.memset(E2[:, :, z:W4], 0.0)
                    nc.gpsimd.memset(O2[:, :, z:W4], 0.0)

                # down2 -> a2
                dg2 = dgp.tile([128, NB * 12, 128], BF16, tag="dg")
                nc.sync.dma_start(dg2[:], diag_d[:, NB * 12:2 * NB * 12, :])
                a2 = amidp.tile([128, NB, W5], BF16, tag="amid")
                for b in range(NB):
                    downconv(b, E2, O2, W5, dg2, a2)
                if first:
                    nc.gpsimd.memset(a2[:, :, 0:0 - s5], 0.0)
                if last:
                    z = (T - 4) - s5
                    nc.gpsimd.memset(a2[:, :, z:W5], 0.0)

                # conv2 (+bias2 in evict) + residual -> out (bf16)
                Lo = min(L, TOUT - t0)
                outt = outp.tile([128, NB, L], BF16, tag="outt")
                for o in range(NB):
                    c0 = 0
                    while c0 < Lo:
                        n = min(CHUNK, Lo - c0)
                        ps = dpsp.tile([128, CHUNK], F32, tag="dps")
                        for idx, (ib, k) in enumerate(
                            (ib, k) for ib in range(NB) for k in range(3)
                        ):
                            nc.tensor.matmul(
                                ps[:, :n], w2t[:, k, ib, o, :],
                                a2[:, ib, c0 + k:c0 + k + n],
                                start=(idx == 0), stop=(idx == 11),
                            )
                        rt = tmpp.tile([128, CHUNK], BF16, tag="tmp")
                        nc.scalar.activation(
                            rt[:, :n], ps[:, :n], AF.Identity,
                            bias=sc_t[:, o, 57:58], scale=1.0,
                        )
                        nc.gpsimd.tensor_add(
                            outt[:, o, c0:c0 + n], rt[:, :n],
                            xt[:, o, 10 + c0:10 + c0 + n],
                        )
                        c0 += n
                nc.sync.dma_start(out_d[:, :, t0:t0 + Lo], outt[:, :, 0:Lo])
    nc.finalize()
    return nc


def _prep_host(x, up_w1, down_w1, alpha1, beta1, up_w2, down_w2, alpha2, beta2,
               c1_w, c1_b, c2_w, c2_b):
    bf = ml_dtypes.bfloat16
    B = x.shape[0]

    def dense_wt(w):
        out = np.empty((128, 3, NB, NB, 128), np.float32)
        wr = w.reshape(NB, 128, NB, 128, 3)  # o, co, i, ci, k
        out[:] = wr.transpose(3, 4, 2, 0, 1)  # (ci, k, i, o, co)
        return out.astype(bf)

    w1t = dense_wt(c1_w)
    w2t = dense_wt(c2_w)

    sc = np.zeros((128, NB, 64), np.float32)
    cidx = np.arange(C)
    for s, (up_w, down_w, alpha, beta) in enumerate(
        ((up_w1, down_w1, alpha1, beta1), (up_w2, down_w2, alpha2, beta2))
    ):
        off = s * S2
        a2v = 2.0 * np.exp(alpha)
        inv2b = 1.0 / (2.0 * np.exp(beta) + 1e-9)
        for b in range(NB):
            cs = cidx[b * 128:(b + 1) * 128]
            for k in range(6):
                sc[:, b, off + k] = up_w[2 * cs, k]
                sc[:, b, off + 6 + k] = up_w[2 * cs + 1, k]
                sc[:, b, off + 12 + k] = down_w[cs, 2 * k]
                sc[:, b, off + 18 + k] = down_w[cs, 2 * k + 1]
            sc[:, b, off + 24] = a2v[cs]
            sc[:, b, off + 25] = np.pi / 2 - a2v[cs] * inv2b[cs]
            sc[:, b, off + 26] = -inv2b[cs]
            sc[:, b, off + 27] = inv2b[cs]
    for b in range(NB):
        cs = cidx[b * 128:(b + 1) * 128]
        sc[:, b, 56] = c1_b[cs]
        sc[:, b, 57] = c2_b[cs]

    diag = np.zeros((128, 2 * NB * 12, 128), np.float32)
    for s, down_w in enumerate((down_w1, down_w2)):
        for b in range(NB):
            cs = cidx[b * 128:(b + 1) * 128]
            for r in range(6):
                i0 = s * NB * 12 + b * 12
                diag[np.arange(128), i0 + r, np.arange(128)] = down_w[cs, 2 * r]
                diag[np.arange(128), i0 + 6 + r, np.arange(128)] = down_w[cs, 2 * r + 1]
    diag = diag.astype(bf)

    in_maps = []
    for bi in range(B):
        xpad = np.zeros((C, T + 2 * PAD), np.float32)
        xpad[:, PAD:PAD + T] = x[bi]
        xp = np.ascontiguousarray(
            xpad.reshape(NB, 128, T + 2 * PAD).transpose(1, 0, 2)
        ).astype(bf)
        in_maps.append({
            "xp": xp, "w1t": w1t, "w2t": w2t, "sc": sc.astype(np.float32),
            "diag": diag,
        })
    return in_maps


_NC_CACHE = None


def _install_profile_hook():
    import types

    try:
        from antenv.axon_hooks import get_axon_ntff_profile_hook  # noqa: F401
        return
    except ImportError:
        pass
    try:
        import antenv
        mod = types.ModuleType("antenv.axon_hooks")
        _state = {"hook": None}
        mod.set_axon_ntff_profile_hook = lambda h: _state.__setitem__("hook", h)
        mod.get_axon_ntff_profile_hook = lambda: _state["hook"]
        sys.modules["antenv.axon_hooks"] = mod
        antenv.axon_hooks = mod
        if "/root/.axon_site" not in sys.path:
            sys.path.insert(0, "/root/.axon_site")
        from trn_agent_boot.trn_boot import _ntff_profile_via_ctypes
        mod.set_axon_ntff_profile_hook(
            _ntff_profile_via_ctypes("/opt/axon/libaxon_pjrt.so"))
    except Exception as e:
        print(f"profile hook install failed: {e}")


def kernel(**inputs):
    global _NC_CACHE, LAST_EXEC_NS, LAST_PROFILE
    import os

    args = {k: np.asarray(v) for k, v in inputs.items()}
    in_maps = _prep_host(**args)
    if _NC_CACHE is None:
        _NC_CACHE = build_graph()
    nc = _NC_CACHE
    trace = bool(os.environ.get("KERNEL_TRACE"))
    kw = {}
    if trace:
        _install_profile_hook()
        kw["tmpdir"] = os.environ.get("KERNEL_TRACE_DIR", "/tmp/ktrace")
        os.makedirs(kw["tmpdir"], exist_ok=True)
    res = run_bass_kernel_spmd(
        nc, in_maps, core_ids=list(range(8)), trace=trace, **kw,
    )
    LAST_EXEC_NS = res.exec_time_ns
    LAST_PROFILE = res.profile_json
    B = len(in_maps)
    out = np.empty((B, C, TOUT), np.float32)
    for bi in range(B):
        o = np.asarray(res.results[bi]["out"]).astype(np.float32)
        out[bi] = o.transpose(1, 0, 2).reshape(C, TOUT)
    return out



# revision 3
# speedup vs baseline: 1.4064x; 1.4064x over previous
"""AMPBlock0 (BigVGAN) Trainium2 kernel: B=8 data-parallel over 8 NeuronCores.

Per core: x (512, 8192) f32 -> out (512, 8188) f32
  a1 = down1(snake1(up1(x)))       # polyphase up x2, SnakeBeta, stride-2 lowpass
  c1 = conv1d_3tap(a1) + b1
  a2 = down2(snake2(up2(c1)))
  out = conv1d_3tap(a2) + b2 + x[:, :8188]

Layout: channels on partitions (4 blocks x 128), time on free axis.
Time tiled (L=992) with halos; all widths <= 1024 so PE sweeps use 2 PSUM
chunks. bf16 storage/matmul, f32 PSUM.

Engine split (v3):
  PE   = down convs (12 diag taps, stationary-outer) + dense convs
         (stationary-outer) + residual via identity matmul into conv2 PSUM
  DVE  = up-conv E phase (TS dual-scalar tap0 + 5 STT), O tap5 product,
         snake combines (STT)
  ACT  = up-conv O phase products (tap0 w/ inv2b bias + taps1-4), cos
         (Sin w/ scale+bias), PSUM evictions (w/ dense bias fusion)
  Pool = up-conv O phase adds (tree), boundary memsets
"""

import sys

if "/opt/trn_rl_repo" not in sys.path:
    sys.path.insert(0, "/opt/trn_rl_repo")

import numpy as np
import ml_dtypes

import concourse.bacc as bacc
import concourse.mybir as mybir
import concourse.tile as tile
from concourse.bass_utils import run_bass_kernel_spmd

BF16 = mybir.dt.bfloat16
F32 = mybir.dt.float32
AF = mybir.ActivationFunctionType
ALU = mybir.AluOpType

T = 8192
C = 512
NB = 4
L = 992
TOUT = T - 4
NT = (TOUT + L - 1) // L  # 9 tiles; last covers 252
PAD = 16
CHUNK = 512

# sc columns (128, NB, 64) f32; stage offset S2=28
# 0-5 we, 6-11 wo, 12-17 d_o, 18-23 d_e, 24 scaleA, 25 biasS, 26 ninv2b, 27 inv2b
# 56 bias1(cout), 57 bias2(cout)
S2 = 28

LAST_EXEC_NS = None
LAST_PROFILE = None


def _chunks(width):
    out, c0 = [], 0
    while c0 < width:
        out.append((c0, min(CHUNK, width - c0)))
        c0 += CHUNK
    return out


def build_graph():
    nc = bacc.Bacc()
    xp_d = nc.declare_dram_parameter("xp", [128, NB, T + 2 * PAD], BF16, isOutput=False)
    w1t_d = nc.declare_dram_parameter("w1t", [128, 3, NB, NB, 128], BF16, isOutput=False)
    w2t_d = nc.declare_dram_parameter("w2t", [128, 3, NB, NB, 128], BF16, isOutput=False)
    sc_d = nc.declare_dram_parameter("sc", [128, NB, 64], F32, isOutput=False)
    # diag: 2 stages x NB x 12 taps, then identity at row 96
    diag_d = nc.declare_dram_parameter("diag", [128, 2 * NB * 12 + 1, 128], BF16,
                                       isOutput=False)
    out_d = nc.declare_dram_parameter("out", [128, NB, TOUT], BF16, isOutput=True)

    with tile.TileContext(nc) as tc:
        with (
            tc.tile_pool(name="const", bufs=1) as constp,
            tc.tile_pool(name="xta", bufs=2) as xtap,
            tc.tile_pool(name="xtb", bufs=2) as xtbp,
            tc.tile_pool(name="eacc", bufs=2) as eaccp,
            tc.tile_pool(name="ofin", bufs=2) as ofinp,
            tc.tile_pool(name="prod", bufs=2) as prodp,
            tc.tile_pool(name="cos", bufs=2) as cosp,
            tc.tile_pool(name="sEt", bufs=2) as sEp,
            tc.tile_pool(name="sOt", bufs=2) as sOp,
            tc.tile_pool(name="amid", bufs=2) as amidp,
            tc.tile_pool(name="c1t", bufs=2) as c1p,
            tc.tile_pool(name="outt", bufs=2) as outp,
            tc.tile_pool(name="dps", bufs=2, space="PSUM") as dpsp,
            tc.tile_pool(name="wps", bufs=2, space="PSUM") as wpsp,
        ):
            w1t = constp.tile([128, 3, NB, NB, 128], BF16)
            nc.sync.dma_start(w1t[:], w1t_d[:])
            w2t = constp.tile([128, 3, NB, NB, 128], BF16)
            nc.sync.dma_start(w2t[:], w2t_d[:])
            sc_t = constp.tile([128, NB, 64], F32)
            nc.sync.dma_start(sc_t[:], sc_d[:])
            dg = constp.tile([128, 2 * NB * 12 + 1, 128], BF16)
            nc.sync.dma_start(dg[:], diag_d[:])
            IDT = 2 * NB * 12  # identity stationary row

            def upsnake(b, xa, xb, width, off, sE, sO):
                """Up-conv (both phases) + snake for block b -> sE, sO tiles.
                E phase on DVE (TS dual + 5 STT); O phase products on ACT,
                adds on Pool; cos on ACT; combines on DVE."""
                # --- E phase: DVE chain
                e0 = eaccp.tile([128, width], BF16, tag="eacc", name="e0")
                nc.vector.tensor_scalar(e0[:], xa[:, b, 0:width],
                                        sc_t[:, b, off + 0:off + 1],
                                        sc_t[:, b, off + 27:off + 28],
                                        ALU.mult, ALU.add)
                cur = e0
                for k in range(1, 5):
                    nxt = eaccp.tile([128, width], BF16, tag="eacc", name="en")
                    nc.vector.scalar_tensor_tensor(
                        nxt[:], xa[:, b, k:k + width],
                        sc_t[:, b, off + k:off + k + 1], cur[:],
                        ALU.mult, ALU.add)
                    cur = nxt
                efin = ofinp.tile([128, width], BF16, tag="efin", name="efin")
                nc.vector.scalar_tensor_tensor(
                    efin[:], xa[:, b, 5:5 + width],
                    sc_t[:, b, off + 5:off + 6], cur[:],
                    ALU.mult, ALU.add)
                # --- O phase: ACT products (tap0 carries inv2b bias), tap5 on DVE
                prods = []
                for k in range(5):
                    p = prodp.tile([128, width], BF16, tag=f"p{k}", name="p")
                    if k == 0:
                        nc.scalar.activation(p[:], xb[:, b, 0:width], AF.Identity,
                                             bias=sc_t[:, b, off + 27:off + 28],
                                             scale=sc_t[:, b, off + 6:off + 7])
                    else:
                        nc.scalar.activation(p[:], xb[:, b, k:k + width], AF.Identity,
                                             bias=0.0,
                                             scale=sc_t[:, b, off + 6 + k:off + 7 + k])
                    prods.append(p)
                p5 = prodp.tile([128, width], BF16, tag="p5", name="p5")
                nc.vector.tensor_scalar(p5[:], xb[:, b, 5:5 + width],
                                        sc_t[:, b, off + 11:off + 12], None, ALU.mult)
                prods.append(p5)
                # Pool add tree: ((p0+p1)+(p2+p3)) + (p4+p5)
                q01 = prodp.tile([128, width], BF16, tag="q01", name="q01")
                nc.gpsimd.tensor_add(q01[:], prods[0][:], prods[1][:])
                q23 = prodp.tile([128, width], BF16, tag="q23", name="q23")
                nc.gpsimd.tensor_add(q23[:], prods[2][:], prods[3][:])
                q45 = prodp.tile([128, width], BF16, tag="q45", name="q45")
                nc.gpsimd.tensor_add(q45[:], prods[4][:], prods[5][:])
                q03 = prodp.tile([128, width], BF16, tag="q03", name="q03")
                nc.gpsimd.tensor_add(q03[:], q01[:], q23[:])
                ofin = ofinp.tile([128, width], BF16, tag="ofin", name="ofin")
                nc.gpsimd.tensor_add(ofin[:], q03[:], q45[:])
                # --- cos on ACT: Sin(scaleA*fin + biasS)
                cosE = cosp.tile([128, width], BF16, tag="cosE", name="cosE")
                nc.scalar.activation(cosE[:], efin[:], AF.Sin,
                                     bias=sc_t[:, b, off + 25:off + 26],
                                     scale=sc_t[:, b, off + 24:off + 25])
                cosO = cosp.tile([128, width], BF16, tag="cosO", name="cosO")
                nc.scalar.activation(cosO[:], ofin[:], AF.Sin,
                                     bias=sc_t[:, b, off + 25:off + 26],
                                     scale=sc_t[:, b, off + 24:off + 25])
                # --- combine on DVE: s = cos*(-inv2b) + fin
                nc.vector.scalar_tensor_tensor(
                    sE[:], cosE[:], sc_t[:, b, off + 26:off + 27], efin[:],
                    ALU.mult, ALU.add)
                nc.vector.scalar_tensor_tensor(
                    sO[:], cosO[:], sc_t[:, b, off + 26:off + 27], ofin[:],
                    ALU.mult, ALU.add)

            def downconv(b, sE, sO, width, stage, dst):
                """12-tap two-phase down conv on PE, stationary-outer."""
                chs = _chunks(width)
                pss = []
                for ci, (c0, n) in enumerate(chs):
                    ps = wpsp.tile([128, CHUNK], F32, tag=f"wps{ci}", name="ps")
                    pss.append(ps)
                base = stage * NB * 12 + b * 12
                for r in range(6):
                    for ci, (c0, n) in enumerate(chs):
                        nc.tensor.matmul(pss[ci][:, :n], dg[:, base + r, :],
                                         sO[:, c0 + r:c0 + r + n],
                                         start=(r == 0), stop=False)
                for r in range(6):
                    last = r == 5
                    for ci, (c0, n) in enumerate(chs):
                        nc.tensor.matmul(pss[ci][:, :n], dg[:, base + 6 + r, :],
                                         sE[:, c0 + r + 1:c0 + r + 1 + n],
                                         start=False, stop=last)
                for ci, (c0, n) in enumerate(chs):
                    nc.scalar.copy(dst[:, b, c0:c0 + n], pss[ci][:, :n])

            for i in range(NT):
                t0 = i * L
                Lo = min(L, TOUT - t0)
                first, last = i == 0, i == NT - 1
                W1, s1 = Lo + 21, t0 - 8
                W2, s2 = Lo + 15, t0 - 6
                W3, s3 = Lo + 13, t0 - 5
                W4, s4 = Lo + 8, t0 - 3
                W5, s5 = Lo + 2, t0 - 1
                Wx, sx = Lo + 26, t0 - 10

                xa = xtap.tile([128, NB, Wx], BF16, name="xa")
                nc.sync.dma_start(xa[:], xp_d[:, :, sx + PAD:sx + PAD + Wx])
                xb = xtbp.tile([128, NB, Wx], BF16, name="xb")
                nc.sync.dma_start(xb[:], xp_d[:, :, sx + PAD:sx + PAD + Wx])

                # ---- stage 1: up + snake -> sE1/sO1 per block; down -> a1
                a1 = amidp.tile([128, NB, W2], BF16, tag="amid", name="a1")
                for b in range(NB):
                    sE = sEp.tile([128, W1], BF16, tag="sE", name="sE")
                    sO = sOp.tile([128, W1], BF16, tag="sO", name="sO")
                    upsnake(b, xa, xb, W1, 0, sE, sO)
                    if first:
                        nc.gpsimd.memset(sE[:, 0:1 - s1], 0.0)
                        nc.gpsimd.memset(sO[:, 0:1 - s1], 0.0)
                    if last:
                        z = (T - 1) - s1
                        nc.gpsimd.memset(sE[:, z:W1], 0.0)
                        nc.gpsimd.memset(sO[:, z:W1], 0.0)
                    downconv(b, sE, sO, W2, 0, a1)
                if first:
                    nc.gpsimd.memset(a1[:, :, 0:0 - s2], 0.0)
                if last:
                    z = (T - 2) - s2
                    nc.gpsimd.memset(a1[:, :, z:W2], 0.0)

                # ---- conv1 -> c1 (+bias1), stationary-outer
                c1 = c1p.tile([128, NB, W3], BF16, tag="c1", name="c1")
                chs3 = _chunks(W3)
                for o in range(NB):
                    pss = []
                    for ci, (c0, n) in enumerate(chs3):
                        ps = dpsp.tile([128, CHUNK], F32, tag=f"dps{ci}", name="ps")
                        pss.append(ps)
                    for idx, (ib, k) in enumerate(
                        (ib, k) for ib in range(NB) for k in range(3)
                    ):
                        for ci, (c0, n) in enumerate(chs3):
                            nc.tensor.matmul(pss[ci][:, :n], w1t[:, k, ib, o, :],
                                             a1[:, ib, c0 + k:c0 + k + n],
                                             start=(idx == 0), stop=(idx == 11))
                    for ci, (c0, n) in enumerate(chs3):
                        nc.scalar.activation(c1[:, o, c0:c0 + n], pss[ci][:, :n],
                                             AF.Identity, bias=sc_t[:, o, 56:57],
                                             scale=1.0)
                if first:
                    nc.gpsimd.memset(c1[:, :, 0:0 - s3], 0.0)
                if last:
                    z = (T - 2) - s3
                    nc.gpsimd.memset(c1[:, :, z:W3], 0.0)

                # ---- stage 2: up + snake (input c1) -> down -> a2
                a2 = amidp.tile([128, NB, W5], BF16, tag="amid", name="a2")
                for b in range(NB):
                    sE = sEp.tile([128, W4], BF16, tag="sE", name="sE2")
                    sO = sOp.tile([128, W4], BF16, tag="sO", name="sO2")
                    upsnake(b, c1, c1, W4, S2, sE, sO)
                    if first:
                        nc.gpsimd.memset(sE[:, 0:1 - s4], 0.0)
                        nc.gpsimd.memset(sO[:, 0:1 - s4], 0.0)
                    if last:
                        z = (T - 3) - s4
                        nc.gpsimd.memset(sE[:, z:W4], 0.0)
                        nc.gpsimd.memset(sO[:, z:W4], 0.0)
                    downconv(b, sE, sO, W5, 1, a2)
                if first:
                    nc.gpsimd.memset(a2[:, :, 0:0 - s5], 0.0)
                if last:
                    z = (T - 4) - s5
                    nc.gpsimd.memset(a2[:, :, z:W5], 0.0)

                # ---- conv2 (+bias2) + residual (identity matmul) -> out
                outt = outp.tile([128, NB, Lo], BF16, tag="outt", name="outt")
                chso = _chunks(Lo)
                for o in range(NB):
                    pss = []
                    for ci, (c0, n) in enumerate(chso):
                        ps = dpsp.tile([128, CHUNK], F32, tag=f"dps{ci}", name="ps2")
                        pss.append(ps)
                    for idx, (ib, k) in enumerate(
                        (ib, k) for ib in range(NB) for k in range(3)
                    ):
                        for ci, (c0, n) in enumerate(chso):
                            nc.tensor.matmul(pss[ci][:, :n], w2t[:, k, ib, o, :],
                                             a2[:, ib, c0 + k:c0 + k + n],
                                             start=(idx == 0), stop=False)
                    # residual: += I * x
                    for ci, (c0, n) in enumerate(chso):
                        nc.tensor.matmul(pss[ci][:, :n], dg[:, IDT, :],
                                         xb[:, o, 10 + c0:10 + c0 + n],
                                         start=False, stop=True)
                    for ci, (c0, n) in enumerate(chso):
                        nc.scalar.activation(outt[:, o, c0:c0 + n], pss[ci][:, :n],
                                             AF.Identity, bias=sc_t[:, o, 57:58],
                                             scale=1.0)
                nc.sync.dma_start(out_d[:, :, t0:t0 + Lo], outt[:, :, 0:Lo])
    nc.finalize()
    return nc


def _prep_host(x, up_w1, down_w1, alpha1, beta1, up_w2, down_w2, alpha2, beta2,
               c1_w, c1_b, c2_w, c2_b):
    bf = ml_dtypes.bfloat16
    B = x.shape[0]

    def dense_wt(w):
        out = np.empty((128, 3, NB, NB, 128), np.float32)
        wr = w.reshape(NB, 128, NB, 128, 3)  # o, co, i, ci, k
        out[:] = wr.transpose(3, 4, 2, 0, 1)  # (ci, k, i, o, co)
        return out.astype(bf)

    w1t = dense_wt(c1_w)
    w2t = dense_wt(c2_w)

    sc = np.zeros((128, NB, 64), np.float32)
    cidx = np.arange(C)
    for s, (up_w, down_w, alpha, beta) in enumerate(
        ((up_w1, down_w1, alpha1, beta1), (up_w2, down_w2, alpha2, beta2))
    ):
        off = s * S2
        a2v = 2.0 * np.exp(alpha)
        inv2b = 1.0 / (2.0 * np.exp(beta) + 1e-9)
        for b in range(NB):
            cs = cidx[b * 128:(b + 1) * 128]
            for k in range(6):
                sc[:, b, off + k] = up_w[2 * cs, k]
                sc[:, b, off + 6 + k] = up_w[2 * cs + 1, k]
                sc[:, b, off + 12 + k] = down_w[cs, 2 * k]
                sc[:, b, off + 18 + k] = down_w[cs, 2 * k + 1]
            sc[:, b, off + 24] = a2v[cs]
            sc[:, b, off + 25] = np.pi / 2 - a2v[cs] * inv2b[cs]
            sc[:, b, off + 26] = -inv2b[cs]
            sc[:, b, off + 27] = inv2b[cs]
    for b in range(NB):
        cs = cidx[b * 128:(b + 1) * 128]
        sc[:, b, 56] = c1_b[cs]
        sc[:, b, 57] = c2_b[cs]

    diag = np.zeros((128, 2 * NB * 12 + 1, 128), np.float32)
    for s, down_w in enumerate((down_w1, down_w2)):
        for b in range(NB):
            cs = cidx[b * 128:(b + 1) * 128]
            for r in range(6):
                i0 = s * NB * 12 + b * 12
                diag[np.arange(128), i0 + r, np.arange(128)] = down_w[cs, 2 * r]
                diag[np.arange(128), i0 + 6 + r, np.arange(128)] = down_w[cs, 2 * r + 1]
    diag[np.arange(128), 2 * NB * 12, np.arange(128)] = 1.0
    diag = diag.astype(bf)

    in_maps = []
    for bi in range(B):
        xpad = np.zeros((C, T + 2 * PAD), np.float32)
        xpad[:, PAD:PAD + T] = x[bi]
        xp = np.ascontiguousarray(
            xpad.reshape(NB, 128, T + 2 * PAD).transpose(1, 0, 2)
        ).astype(bf)
        in_maps.append({
            "xp": xp, "w1t": w1t, "w2t": w2t, "sc": sc.astype(np.float32),
            "diag": diag,
        })
    return in_maps


_NC_CACHE = None


def _install_profile_hook():
    import types

    try:
        from antenv.axon_hooks import get_axon_ntff_profile_hook  # noqa: F401
        return
    except ImportError:
        pass
    try:
        import antenv
        mod = types.ModuleType("antenv.axon_hooks")
        _state = {"hook": None}
        mod.set_axon_ntff_profile_hook = lambda h: _state.__setitem__("hook", h)
        mod.get_axon_ntff_profile_hook = lambda: _state["hook"]
        sys.modules["antenv.axon_hooks"] = mod
        antenv.axon_hooks = mod
        if "/root/.axon_site" not in sys.path:
            sys.path.insert(0, "/root/.axon_site")
        from trn_agent_boot.trn_boot import _ntff_profile_via_ctypes
        mod.set_axon_ntff_profile_hook(
            _ntff_profile_via_ctypes("/opt/axon/libaxon_pjrt.so"))
    except Exception as e:
        print(f"profile hook install failed: {e}")


def kernel(**inputs):
    global _NC_CACHE, LAST_EXEC_NS, LAST_PROFILE
    import os

    args = {k: np.asarray(v) for k, v in inputs.items()}
    in_maps = _prep_host(**args)
    if _NC_CACHE is None:
        _NC_CACHE = build_graph()
    nc = _NC_CACHE
    trace = bool(os.environ.get("KERNEL_TRACE"))
    kw = {}
    if trace:
        _install_profile_hook()
        kw["tmpdir"] = os.environ.get("KERNEL_TRACE_DIR", "/tmp/ktrace")
        os.makedirs(kw["tmpdir"], exist_ok=True)
    res = run_bass_kernel_spmd(
        nc, in_maps, core_ids=list(range(8)), trace=trace, **kw,
    )
    LAST_EXEC_NS = res.exec_time_ns
    LAST_PROFILE = res.profile_json
    B = len(in_maps)
    out = np.empty((B, C, TOUT), np.float32)
    for bi in range(B):
        o = np.asarray(res.results[bi]["out"]).astype(np.float32)
        out[bi] = o.transpose(1, 0, 2).reshape(C, TOUT)
    return out


# revision 8
# speedup vs baseline: 2.9468x; 2.0953x over previous
"""AMPBlock0 (BigVGAN) Trainium2 kernel: B=8 data-parallel over 8 NeuronCores.

Per core: x (512, 8192) f32 -> out (512, 8188) f32
  a1 = down1(snake1(up1(x)))       # polyphase up x2, SnakeBeta, stride-2 lowpass
  c1 = conv1d_3tap(a1) + b1
  a2 = down2(snake2(up2(c1)))
  out = conv1d_3tap(a2) + b2 + x[:, :8188]

Layout: channels on partitions (4 blocks x 128), time on free axis.
Time tiled (L=992) with halos; widths <= 1024 (2 PSUM chunks).

Engine split (v4 - fp8 DoubleRow):
  PE   = ALL convs as fp8e4 DoubleRow matmuls packing 2 taps (or 2 input
         blocks) per instruction: up (3 DR/phase), down (6 DR, E/O pair
         via shifted dual-row s tile), dense (6 DR via block pairs);
         residual via diag(128) bf16 matmul into conv2 PSUM.
  ACT  = cos = Sin(scale*psum + pi/2) straight from up PSUM, E-phase fin
         eviction (+inv2b), a eviction (->fp8), c1 row0 eviction, c2
         eviction.
  DVE  = O-phase fin eviction (TS from PSUM), snake combines (STT ->
         fp8 dual-row s tile), c1 row1 (shifted) eviction.
  Pool = boundary memsets only.

Weight scaling for fp8 range: up x8, down x8, dense x128 (undone at
PSUM eviction; residual rides conv2 PSUM as diag(128)).
"""

import sys

if "/opt/trn_rl_repo" not in sys.path:
    sys.path.insert(0, "/opt/trn_rl_repo")

import numpy as np
import ml_dtypes

import concourse.bacc as bacc
import concourse.mybir as mybir
import concourse.tile as tile
from concourse.bass_utils import run_bass_kernel_spmd

BF16 = mybir.dt.bfloat16
F8 = mybir.dt.float8e4
F32 = mybir.dt.float32
AF = mybir.ActivationFunctionType
ALU = mybir.AluOpType
PM = mybir.MatmulPerfMode

T = 8192
C = 512
NB = 4
L = 992
TOUT = T - 4
NT = (TOUT + L - 1) // L
PAD = 16
CHUNK = 512

US = 8.0     # up weight scale
DS = 8.0     # down weight scale
WS = 128.0   # dense weight scale

# sc columns (128, NB, 64) f32; stage offset S2=28
# 24: 2*e^alpha / US (cos scale on psum), 26: -inv2b, 27: inv2b
# 56 bias1(cout), 57 bias2(cout)
S2 = 28

LAST_EXEC_NS = None
LAST_PROFILE = None


def _chunks(width):
    out, c0 = [], 0
    while c0 < width:
        out.append((c0, min(CHUNK, width - c0)))
        c0 += CHUNK
    return out


def build_graph():
    nc = bacc.Bacc()
    x8_d = nc.declare_dram_parameter("x8", [128, NB, T + 2 * PAD], F8, isOutput=False)
    xb_d = nc.declare_dram_parameter("xb", [128, NB, T + 2 * PAD], BF16, isOutput=False)
    # up stationaries: (stage, block, phase, pair) -> [2, 128]
    up8_d = nc.declare_dram_parameter("up8", [128, 2 * NB * 2 * 3, 2, 128], F8,
                                      isOutput=False)
    # down stationaries: (stage, block, r) -> [2, 128]; row0=d_e[r], row1=d_o[r]
    dn8_d = nc.declare_dram_parameter("dn8", [128, 2 * NB * 6, 2, 128], F8,
                                      isOutput=False)
    # dense stationaries: (k, P, i, o) -> [128]; per conv
    w18_d = nc.declare_dram_parameter("w18", [128, 3, 2, 2, NB, 128], F8,
                                      isOutput=False)
    w28_d = nc.declare_dram_parameter("w28", [128, 3, 2, 2, NB, 128], F8,
                                      isOutput=False)
    idr_d = nc.declare_dram_parameter("idr", [128, 128], BF16, isOutput=False)
    sc_d = nc.declare_dram_parameter("sc", [128, NB, 64], F32, isOutput=False)
    out_d = nc.declare_dram_parameter("out", [128, NB, TOUT], BF16, isOutput=True)

    with tile.TileContext(nc) as tc:
        with (
            tc.tile_pool(name="const", bufs=1) as constp,
            tc.tile_pool(name="x8t", bufs=2) as x8p,
            tc.tile_pool(name="xbt", bufs=2) as xbp,
            tc.tile_pool(name="fin", bufs=3) as finp,
            tc.tile_pool(name="cost", bufs=3) as cosp,
            tc.tile_pool(name="st", bufs=3) as stp,
            tc.tile_pool(name="amid", bufs=2) as amidp,
            tc.tile_pool(name="c1t", bufs=2) as c1p,
            tc.tile_pool(name="outt", bufs=2) as outp,
            tc.tile_pool(name="ups", bufs=2, space="PSUM") as upsp,
            tc.tile_pool(name="dns", bufs=2, space="PSUM") as dnsp,
            tc.tile_pool(name="dss", bufs=2, space="PSUM") as dssp,
        ):
            up8 = constp.tile([128, 2 * NB * 2 * 3, 2, 128], F8)
            nc.sync.dma_start(up8[:], up8_d[:])
            dn8 = constp.tile([128, 2 * NB * 6, 2, 128], F8)
            nc.sync.dma_start(dn8[:], dn8_d[:])
            w18 = constp.tile([128, 3, 2, 2, NB, 128], F8)
            nc.sync.dma_start(w18[:], w18_d[:])
            w28 = constp.tile([128, 3, 2, 2, NB, 128], F8)
            nc.sync.dma_start(w28[:], w28_d[:])
            idr = constp.tile([128, 128], BF16)
            nc.sync.dma_start(idr[:], idr_d[:])
            sc_t = constp.tile([128, NB, 64], F32)
            nc.sync.dma_start(sc_t[:], sc_d[:])

            HALF_PI = float(np.pi / 2)

            def upsnake(b, src, width, stage, S):
                """Up-conv (fp8 DR on PE) + snake for block b -> dual-row S.
                src: [128, NB, 2, Wsrc] fp8 (row1 = row0 shifted +1).
                S: [128, 2, width] fp8; row0[j] = sE[j+1], row1[j] = sO[j]."""
                off = stage * S2
                for phase in range(2):
                    fin = finp.tile([128, width], BF16, tag=f"fin{phase}", name="fin")
                    cosx = cosp.tile([128, width], BF16, tag=f"cos{phase}", name="cosx")
                    for c0, n in _chunks(width):
                        ps = upsp.tile([128, CHUNK], F32, tag=f"up{phase}", name="ups")
                        for p in range(3):
                            si = ((stage * NB + b) * 2 + phase) * 3 + p
                            nc.tensor.matmul(
                                ps[:, :n], up8[:, si, :, :],
                                src[:, b, :, 2 * p + c0:2 * p + c0 + n],
                                start=(p == 0), stop=(p == 2),
                                perf_mode=PM.DoubleRow)
                        # cos = Sin((2 e^a / US) * psum + pi/2)  [ACT]
                        nc.scalar.activation(cosx[:, c0:c0 + n], ps[:, :n], AF.Sin,
                                             bias=sc_t[:, b, off + 25:off + 26],
                                             scale=sc_t[:, b, off + 24:off + 25])
                        # fin = psum/US + inv2b
                        if phase == 0:
                            nc.scalar.activation(fin[:, c0:c0 + n], ps[:, :n],
                                                 AF.Identity,
                                                 bias=sc_t[:, b, off + 27:off + 28],
                                                 scale=1.0 / US)
                        else:
                            nc.vector.tensor_scalar(fin[:, c0:c0 + n], ps[:, :n],
                                                    1.0 / US,
                                                    sc_t[:, b, off + 27:off + 28],
                                                    ALU.mult, ALU.add)
                    # combine: s = cos * (-inv2b) + fin  [DVE] -> fp8 S rows
                    if phase == 0:  # E: row0[j] = sE[j+1]
                        nc.vector.scalar_tensor_tensor(
                            S[:, 0, 0:width - 1], cosx[:, 1:width],
                            sc_t[:, b, off + 26:off + 27], fin[:, 1:width],
                            ALU.mult, ALU.add)
                    else:  # O: row1[j] = sO[j]
                        nc.vector.scalar_tensor_tensor(
                            S[:, 1, 0:width], cosx[:, 0:width],
                            sc_t[:, b, off + 26:off + 27], fin[:, 0:width],
                            ALU.mult, ALU.add)

            def downconv(b, S, width, stage, dst):
                """12-tap down conv as 6 fp8 DR matmuls; dst fp8 (scale 1/DS)."""
                for c0, n in _chunks(width):
                    ps = dnsp.tile([128, CHUNK], F32, tag="dn", name="dps")
                    for r in range(6):
                        nc.tensor.matmul(ps[:, :n],
                                         dn8[:, (stage * NB + b) * 6 + r, :, :],
                                         S[:, :, c0 + r:c0 + r + n],
                                         start=(r == 0), stop=(r == 5),
                                         perf_mode=PM.DoubleRow)
                    nc.scalar.mul(dst[:, b, c0:c0 + n], ps[:, :n], 1.0 / DS)

            def dense(w8, a, width, bcol, dst_ap_fn, residual_src=None):
                """3-tap dense conv as 6 fp8 DR matmuls per out-block.
                dst_ap_fn(o, c0, n) -> (act_out_ap or None, dve_out_ap or None)"""
                for o in range(NB):
                    for c0, n in _chunks(width):
                        ps = dssp.tile([128, CHUNK], F32, tag="ds", name="dsps")
                        idx = 0
                        for k in range(3):
                            for P in range(2):
                                nc.tensor.matmul(
                                    ps[:, :n], w8[:, k, P, :, o, :],
                                    a[:, 2 * P:2 * P + 2, c0 + k:c0 + k + n],
                                    start=(idx == 0),
                                    stop=(idx == 5 and residual_src is None),
                                    perf_mode=PM.DoubleRow)
                                idx += 1
                        if residual_src is not None:
                            nc.tensor.matmul(ps[:, :n], idr[:, :],
                                             residual_src[:, o, 10 + c0:10 + c0 + n],
                                             start=False, stop=True)
                        dst_ap_fn(o, c0, n, ps)

            for i in range(NT):
                t0 = i * L
                Lo = min(L, TOUT - t0)
                first, last = i == 0, i == NT - 1
                W1, s1 = Lo + 21, t0 - 8
                W2, s2 = Lo + 15, t0 - 6
                W3, s3 = Lo + 13, t0 - 5
                W4, s4 = Lo + 8, t0 - 3
                W5, s5 = Lo + 2, t0 - 1
                Wx, sx = Lo + 26, t0 - 10

                x8t = x8p.tile([128, NB, 2, Wx], F8, name="x8t")
                nc.sync.dma_start(x8t[:, :, 0, :], x8_d[:, :, sx + PAD:sx + PAD + Wx])
                nc.sync.dma_start(x8t[:, :, 1, :],
                                  x8_d[:, :, sx + 1 + PAD:sx + 1 + PAD + Wx])
                xbt = xbp.tile([128, NB, Wx], BF16, name="xbt")
                nc.sync.dma_start(xbt[:], xb_d[:, :, sx + PAD:sx + PAD + Wx])

                # ---- stage 1: up+snake -> S per block -> down -> a1 (fp8)
                a1 = amidp.tile([128, NB, W2], F8, tag="amid", name="a1")
                for b in range(NB):
                    S = stp.tile([128, 2, W1], F8, tag="S", name="S1")
                    upsnake(b, x8t, W1, 0, S)
                    if first:
                        nc.gpsimd.memset(S[:, 0, 0:-s1], 0.0)       # sE[0:1-s1]
                        nc.gpsimd.memset(S[:, 1, 0:1 - s1], 0.0)    # sO[0:1-s1]
                    if last:
                        z = (T - 1) - s1
                        nc.gpsimd.memset(S[:, 0, z - 1:W1], 0.0)
                        nc.gpsimd.memset(S[:, 1, z:W1], 0.0)
                    downconv(b, S, W2, 0, a1)
                if first:
                    nc.gpsimd.memset(a1[:, :, 0:0 - s2], 0.0)
                if last:
                    z = (T - 2) - s2
                    nc.gpsimd.memset(a1[:, :, z:W2], 0.0)

                # ---- conv1 -> c1 dual-row fp8 (+bias1)
                c1 = c1p.tile([128, NB, 2, W3], F8, tag="c1", name="c1")

                def c1_out(o, c0, n, ps):
                    nc.scalar.activation(c1[:, o, 0, c0:c0 + n], ps[:, :n],
                                         AF.Identity, bias=sc_t[:, o, 56:57],
                                         scale=1.0 / WS)
                    if c0 == 0:
                        nc.vector.tensor_scalar(c1[:, o, 1, 0:n - 1], ps[:, 1:n],
                                                1.0 / WS, sc_t[:, o, 56:57],
                                                ALU.mult, ALU.add)
                    else:
                        nc.vector.tensor_scalar(c1[:, o, 1, c0 - 1:c0 + n - 1],
                                                ps[:, 0:n],
                                                1.0 / WS, sc_t[:, o, 56:57],
                                                ALU.mult, ALU.add)

                dense(w18, a1, W3, 56, c1_out)
                if first:
                    nc.gpsimd.memset(c1[:, :, 0, 0:0 - s3], 0.0)
                    if 0 - s3 - 1 > 0:
                        nc.gpsimd.memset(c1[:, :, 1, 0:0 - s3 - 1], 0.0)
                if last:
                    z = (T - 2) - s3
                    nc.gpsimd.memset(c1[:, :, 0, z:W3], 0.0)
                    nc.gpsimd.memset(c1[:, :, 1, z - 1:W3], 0.0)

                # ---- stage 2: up+snake -> down -> a2 (fp8)
                a2 = amidp.tile([128, NB, W5], F8, tag="amid", name="a2")
                for b in range(NB):
                    S = stp.tile([128, 2, W4], F8, tag="S", name="S2")
                    upsnake(b, c1, W4, 1, S)
                    if first:
                        nc.gpsimd.memset(S[:, 0, 0:-s4], 0.0)
                        nc.gpsimd.memset(S[:, 1, 0:1 - s4], 0.0)
                    if last:
                        z = (T - 3) - s4
                        nc.gpsimd.memset(S[:, 0, z - 1:W4], 0.0)
                        nc.gpsimd.memset(S[:, 1, z:W4], 0.0)
                    downconv(b, S, W5, 1, a2)
                if first:
                    nc.gpsimd.memset(a2[:, :, 0:0 - s5], 0.0)
                if last:
                    z = (T - 4) - s5
                    nc.gpsimd.memset(a2[:, :, z:W5], 0.0)

                # ---- conv2 (+bias2) + residual -> out
                outt = outp.tile([128, NB, Lo], BF16, tag="outt", name="outt")

                def c2_out(o, c0, n, ps):
                    nc.scalar.activation(outt[:, o, c0:c0 + n], ps[:, :n],
                                         AF.Identity, bias=sc_t[:, o, 57:58],
                                         scale=1.0 / WS)

                dense(w28, a2, Lo, 57, c2_out, residual_src=xbt)
                nc.sync.dma_start(out_d[:, :, t0:t0 + Lo], outt[:, :, 0:Lo])
    nc.finalize()
    return nc


def _prep_host(x, up_w1, down_w1, alpha1, beta1, up_w2, down_w2, alpha2, beta2,
               c1_w, c1_b, c2_w, c2_b):
    bf = ml_dtypes.bfloat16
    f8 = ml_dtypes.float8_e4m3
    B = x.shape[0]
    ar = np.arange(128)

    def dense_w8(w):
        # [128ci_local, 3k, 2P, 2i, NBo, 128co] ; ci = (2P+i)*128 + ci_local
        out = np.empty((128, 3, 2, 2, NB, 128), np.float32)
        wr = w.reshape(NB, 128, NB, 128, 3)  # o, co, i, ci, k
        for P in range(2):
            for ii in range(2):
                # in-block index 2P+ii -> (ci_local, k, o, co)
                out[:, :, P, ii, :, :] = wr[:, :, 2 * P + ii, :, :].transpose(2, 3, 0, 1)
        return (out * WS).astype(f8)

    w18 = dense_w8(c1_w)
    w28 = dense_w8(c2_w)

    up8 = np.zeros((128, 2 * NB * 2 * 3, 2, 128), np.float32)
    dn8 = np.zeros((128, 2 * NB * 6, 2, 128), np.float32)
    sc = np.zeros((128, NB, 64), np.float32)
    cidx = np.arange(C)
    for s, (up_w, down_w, alpha, beta) in enumerate(
        ((up_w1, down_w1, alpha1, beta1), (up_w2, down_w2, alpha2, beta2))
    ):
        off = s * S2
        a2v = 2.0 * np.exp(alpha)
        inv2b = 1.0 / (2.0 * np.exp(beta) + 1e-9)
        for b in range(NB):
            cs = cidx[b * 128:(b + 1) * 128]
            for phase in range(2):
                for p in range(3):
                    si = ((s * NB + b) * 2 + phase) * 3 + p
                    up8[ar, si, 0, ar] = up_w[2 * cs + phase, 2 * p] * US
                    up8[ar, si, 1, ar] = up_w[2 * cs + phase, 2 * p + 1] * US
            for r in range(6):
                di = (s * NB + b) * 6 + r
                dn8[ar, di, 0, ar] = down_w[cs, 2 * r + 1] * DS  # d_e[r]
                dn8[ar, di, 1, ar] = down_w[cs, 2 * r] * DS      # d_o[r]
            sc[:, b, off + 24] = a2v[cs] / US
            sc[:, b, off + 25] = np.pi / 2
            sc[:, b, off + 26] = -inv2b[cs]
            sc[:, b, off + 27] = inv2b[cs]
    for b in range(NB):
        cs = cidx[b * 128:(b + 1) * 128]
        sc[:, b, 56] = c1_b[cs]
        sc[:, b, 57] = c2_b[cs]

    idr = np.zeros((128, 128), np.float32)
    idr[ar, ar] = WS
    idr = idr.astype(bf)

    up8 = up8.astype(f8)
    dn8 = dn8.astype(f8)

    in_maps = []
    for bi in range(B):
        xpad = np.zeros((C, T + 2 * PAD), np.float32)
        xpad[:, PAD:PAD + T] = x[bi]
        xr = np.ascontiguousarray(
            xpad.reshape(NB, 128, T + 2 * PAD).transpose(1, 0, 2))
        in_maps.append({
            "x8": xr.astype(f8), "xb": xr.astype(bf),
            "up8": up8, "dn8": dn8, "w18": w18, "w28": w28,
            "idr": idr, "sc": sc.astype(np.float32),
        })
    return in_maps


_NC_CACHE = None


def _install_profile_hook():
    import types

    try:
        from antenv.axon_hooks import get_axon_ntff_profile_hook  # noqa: F401
        return
    except ImportError:
        pass
    try:
        import antenv
        mod = types.ModuleType("antenv.axon_hooks")
        _state = {"hook": None}
        mod.set_axon_ntff_profile_hook = lambda h: _state.__setitem__("hook", h)
        mod.get_axon_ntff_profile_hook = lambda: _state["hook"]
        sys.modules["antenv.axon_hooks"] = mod
        antenv.axon_hooks = mod
        if "/root/.axon_site" not in sys.path:
            sys.path.insert(0, "/root/.axon_site")
        from trn_agent_boot.trn_boot import _ntff_profile_via_ctypes
        mod.set_axon_ntff_profile_hook(
            _ntff_profile_via_ctypes("/opt/axon/libaxon_pjrt.so"))
    except Exception as e:
        print(f"profile hook install failed: {e}")


def kernel(**inputs):
    global _NC_CACHE, LAST_EXEC_NS, LAST_PROFILE
    import os

    args = {k: np.asarray(v) for k, v in inputs.items()}
    in_maps = _prep_host(**args)
    if _NC_CACHE is None:
        _NC_CACHE = build_graph()
    nc = _NC_CACHE
    trace = bool(os.environ.get("KERNEL_TRACE"))
    kw = {}
    if trace:
        _install_profile_hook()
        kw["tmpdir"] = os.environ.get("KERNEL_TRACE_DIR", "/tmp/ktrace")
        os.makedirs(kw["tmpdir"], exist_ok=True)
    res = run_bass_kernel_spmd(
        nc, in_maps, core_ids=list(range(8)), trace=trace, **kw,
    )
    LAST_EXEC_NS = res.exec_time_ns
    LAST_PROFILE = res.profile_json
    B = len(in_maps)
    out = np.empty((B, C, TOUT), np.float32)
    for bi in range(B):
        o = np.asarray(res.results[bi]["out"]).astype(np.float32)
        out[bi] = o.transpose(1, 0, 2).reshape(C, TOUT)
    return out


# revision 11
# speedup vs baseline: 3.0393x; 1.0314x over previous
"""AMPBlock0 (BigVGAN) Trainium2 kernel: B=8 data-parallel over 8 NeuronCores.

Per core: x (512, 8192) f32 -> out (512, 8188) f32
  a1 = down1(snake1(up1(x)))       # polyphase up x2, SnakeBeta, stride-2 lowpass
  c1 = conv1d_3tap(a1) + b1
  a2 = down2(snake2(up2(c1)))
  out = conv1d_3tap(a2) + b2 + x[:, :8188]

Layout: channels on partitions (4 blocks x 128), time on free axis.
Time tiled (L=992) with halos; widths <= 1024 (2 PSUM chunks).

Engine split (v4 - fp8 DoubleRow):
  PE   = ALL convs as fp8e4 DoubleRow matmuls packing 2 taps (or 2 input
         blocks) per instruction: up (3 DR/phase), down (6 DR, E/O pair
         via shifted dual-row s tile), dense (6 DR via block pairs);
         residual via diag(128) bf16 matmul into conv2 PSUM.
  ACT  = cos = Sin(scale*psum + pi/2) straight from up PSUM, E-phase fin
         eviction (+inv2b), a eviction (->fp8), c1 row0 eviction, c2
         eviction.
  DVE  = O-phase fin eviction (TS from PSUM), snake combines (STT ->
         fp8 dual-row s tile), c1 row1 (shifted) eviction.
  Pool = boundary memsets only.

Weight scaling for fp8 range: up x8, down x8, dense x128 (undone at
PSUM eviction; residual rides conv2 PSUM as diag(128)).
"""

import sys

if "/opt/trn_rl_repo" not in sys.path:
    sys.path.insert(0, "/opt/trn_rl_repo")

import numpy as np
import ml_dtypes

import concourse.bacc as bacc
import concourse.mybir as mybir
import concourse.tile as tile
from concourse.bass_utils import run_bass_kernel_spmd

BF16 = mybir.dt.bfloat16
F8 = mybir.dt.float8e4
F32 = mybir.dt.float32
AF = mybir.ActivationFunctionType
ALU = mybir.AluOpType
PM = mybir.MatmulPerfMode

T = 8192
C = 512
NB = 4
L = 992
TOUT = T - 4
NT = (TOUT + L - 1) // L
PAD = 16
CHUNK = 512

US = 8.0     # up weight scale
DS = 8.0     # down weight scale
WS = 128.0   # dense weight scale

# sc columns (128, NB, 64) f32; stage offset S2=28
# 24: 2*e^alpha / US (cos scale on psum), 26: -inv2b, 27: inv2b
# 56 bias1(cout), 57 bias2(cout)
S2 = 28

LAST_EXEC_NS = None
LAST_PROFILE = None


def _chunks(width):
    out, c0 = [], 0
    while c0 < width:
        out.append((c0, min(CHUNK, width - c0)))
        c0 += CHUNK
    return out


def build_graph():
    nc = bacc.Bacc()
    x8_d = nc.declare_dram_parameter("x8", [128, NB, T + 2 * PAD], F8, isOutput=False)
    xb_d = nc.declare_dram_parameter("xb", [128, NB, T + 2 * PAD], BF16, isOutput=False)
    # up stationaries: (stage, block, phase, pair) -> [2, 128]
    up8_d = nc.declare_dram_parameter("up8", [128, 2 * NB * 2 * 3, 2, 128], F8,
                                      isOutput=False)
    # down stationaries: (stage, block, r) -> [2, 128]; row0=d_e[r], row1=d_o[r]
    dn8_d = nc.declare_dram_parameter("dn8", [128, 2 * NB * 6, 2, 128], F8,
                                      isOutput=False)
    # dense stationaries: (k, P, i, o) -> [128]; per conv
    w18_d = nc.declare_dram_parameter("w18", [128, 3, 2, 2, NB, 128], F8,
                                      isOutput=False)
    w28_d = nc.declare_dram_parameter("w28", [128, 3, 2, 2, NB, 128], F8,
                                      isOutput=False)
    idr_d = nc.declare_dram_parameter("idr", [128, 128], BF16, isOutput=False)
    sc_d = nc.declare_dram_parameter("sc", [128, NB, 64], F32, isOutput=False)
    out_d = nc.declare_dram_parameter("out", [128, NB, TOUT], BF16, isOutput=True)

    with tile.TileContext(nc) as tc:
        with (
            tc.tile_pool(name="const", bufs=1) as constp,
            tc.tile_pool(name="x8t", bufs=2) as x8p,
            tc.tile_pool(name="xbt", bufs=2) as xbp,
            tc.tile_pool(name="fin", bufs=3) as finp,
            tc.tile_pool(name="cost", bufs=3) as cosp,
            tc.tile_pool(name="st", bufs=2) as stp,
            tc.tile_pool(name="amid", bufs=2) as amidp,
            tc.tile_pool(name="c1t", bufs=2) as c1p,
            tc.tile_pool(name="outt", bufs=2) as outp,
            tc.tile_pool(name="ups", bufs=2, space="PSUM") as upsp,
            tc.tile_pool(name="dns", bufs=2, space="PSUM") as dnsp,
            tc.tile_pool(name="dss", bufs=2, space="PSUM") as dssp,
        ):
            up8 = constp.tile([128, 2 * NB * 2 * 3, 2, 128], F8)
            nc.sync.dma_start(up8[:], up8_d[:])
            dn8 = constp.tile([128, 2 * NB * 6, 2, 128], F8)
            nc.sync.dma_start(dn8[:], dn8_d[:])
            w18 = constp.tile([128, 3, 2, 2, NB, 128], F8)
            nc.sync.dma_start(w18[:], w18_d[:])
            w28 = constp.tile([128, 3, 2, 2, NB, 128], F8)
            nc.sync.dma_start(w28[:], w28_d[:])
            idr = constp.tile([128, 128], BF16)
            nc.sync.dma_start(idr[:], idr_d[:])
            sc_t = constp.tile([128, NB, 64], F32)
            nc.sync.dma_start(sc_t[:], sc_d[:])

            HALF_PI = float(np.pi / 2)

            def upsnake(b, src, width, stage, S):
                """Up-conv (fp8 DR on PE) + snake for block b -> dual-row S.
                src: [128, NB, 2, Wsrc] fp8 (row1 = row0 shifted +1).
                S: [128, 2, width] fp8; row0[j] = sE[j+1], row1[j] = sO[j]."""
                off = stage * S2
                for phase in range(2):
                    fin = finp.tile([128, width], BF16, tag=f"fin{phase}", name="fin")
                    cosx = cosp.tile([128, width], BF16, tag=f"cos{phase}", name="cosx")
                    for c0, n in _chunks(width):
                        ps = upsp.tile([128, CHUNK], F32, tag=f"up{phase}", name="ups")
                        for p in range(3):
                            si = ((stage * NB + b) * 2 + phase) * 3 + p
                            nc.tensor.matmul(
                                ps[:, :n], up8[:, si, :, :],
                                src[:, b, :, 2 * p + c0:2 * p + c0 + n],
                                start=(p == 0), stop=(p == 2),
                                perf_mode=PM.DoubleRow)
                        # cos = Sin((2 e^a / US) * psum + pi/2)  [ACT]
                        nc.scalar.activation(cosx[:, c0:c0 + n], ps[:, :n], AF.Sin,
                                             bias=sc_t[:, b, off + 25:off + 26],
                                             scale=sc_t[:, b, off + 24:off + 25])
                        # fin = psum/US + inv2b
                        if phase == 0:
                            nc.scalar.activation(fin[:, c0:c0 + n], ps[:, :n],
                                                 AF.Identity,
                                                 bias=sc_t[:, b, off + 27:off + 28],
                                                 scale=1.0 / US)
                        else:
                            nc.vector.tensor_scalar(fin[:, c0:c0 + n], ps[:, :n],
                                                    1.0 / US,
                                                    sc_t[:, b, off + 27:off + 28],
                                                    ALU.mult, ALU.add)
                    # combine: s = cos * (-inv2b) + fin  [DVE] -> fp8 S rows
                    if phase == 0:  # E: row0[j] = sE[j+1]
                        nc.vector.scalar_tensor_tensor(
                            S[:, 0, 0:width - 1], cosx[:, 1:width],
                            sc_t[:, b, off + 26:off + 27], fin[:, 1:width],
                            ALU.mult, ALU.add)
                    else:  # O: row1[j] = sO[j]
                        nc.vector.scalar_tensor_tensor(
                            S[:, 1, 0:width], cosx[:, 0:width],
                            sc_t[:, b, off + 26:off + 27], fin[:, 0:width],
                            ALU.mult, ALU.add)

            def downconv(b, S, width, stage, dst):
                """12-tap down conv as 6 fp8 DR matmuls; dst fp8 (scale 1/DS)."""
                for c0, n in _chunks(width):
                    ps = dnsp.tile([128, CHUNK], F32, tag="dn", name="dps")
                    for r in range(6):
                        nc.tensor.matmul(ps[:, :n],
                                         dn8[:, (stage * NB + b) * 6 + r, :, :],
                                         S[:, :, c0 + r:c0 + r + n],
                                         start=(r == 0), stop=(r == 5),
                                         perf_mode=PM.DoubleRow)
                    nc.scalar.mul(dst[:, b, c0:c0 + n], ps[:, :n], 1.0 / DS)

            def dense(w8, a, width, bcol, dst_ap_fn, residual_src=None):
                """3-tap dense conv as 6 fp8 DR matmuls per out-block.
                dst_ap_fn(o, c0, n) -> (act_out_ap or None, dve_out_ap or None)"""
                for o in range(NB):
                    for c0, n in _chunks(width):
                        ps = dssp.tile([128, CHUNK], F32, tag="ds", name="dsps")
                        idx = 0
                        for k in range(3):
                            for P in range(2):
                                nc.tensor.matmul(
                                    ps[:, :n], w8[:, k, P, :, o, :],
                                    a[:, 2 * P:2 * P + 2, c0 + k:c0 + k + n],
                                    start=(idx == 0),
                                    stop=(idx == 5 and residual_src is None),
                                    perf_mode=PM.DoubleRow)
                                idx += 1
                        if residual_src is not None:
                            nc.tensor.matmul(ps[:, :n], idr[:, :],
                                             residual_src[:, o, 10 + c0:10 + c0 + n],
                                             start=False, stop=True)
                        dst_ap_fn(o, c0, n, ps)

            for i in range(NT):
                t0 = i * L
                Lo = min(L, TOUT - t0)
                first, last = i == 0, i == NT - 1
                W1, s1 = Lo + 21, t0 - 8
                W2, s2 = Lo + 15, t0 - 6
                W3, s3 = Lo + 13, t0 - 5
                W4, s4 = Lo + 8, t0 - 3
                W5, s5 = Lo + 2, t0 - 1
                Wx, sx = Lo + 26, t0 - 10

                x8t = x8p.tile([128, NB, 2, Wx], F8, name="x8t")
                nc.sync.dma_start(x8t[:, :, 0, :], x8_d[:, :, sx + PAD:sx + PAD + Wx])
                nc.sync.dma_start(x8t[:, :, 1, :],
                                  x8_d[:, :, sx + 1 + PAD:sx + 1 + PAD + Wx])
                xbt = xbp.tile([128, NB, Wx], BF16, name="xbt")
                nc.sync.dma_start(xbt[:], xb_d[:, :, sx + PAD:sx + PAD + Wx])

                # ---- stage 1: up+snake -> S per block -> down -> a1 (fp8)
                a1 = amidp.tile([128, NB, W2], F8, tag="amid", name="a1")
                Ss = []
                for b in range(NB):
                    S = stp.tile([128, 2, W1], F8, tag=f"S{b}", name="S1")
                    upsnake(b, x8t, W1, 0, S)
                    if first:
                        nc.gpsimd.memset(S[:, 0, 0:-s1], 0.0)       # sE[0:1-s1]
                        nc.gpsimd.memset(S[:, 1, 0:1 - s1], 0.0)    # sO[0:1-s1]
                    if last:
                        z = (T - 1) - s1
                        nc.gpsimd.memset(S[:, 0, z - 1:W1], 0.0)
                        nc.gpsimd.memset(S[:, 1, z:W1], 0.0)
                    Ss.append(S)
                for b in range(NB):
                    downconv(b, Ss[b], W2, 0, a1)
                if first:
                    nc.gpsimd.memset(a1[:, :, 0:0 - s2], 0.0)
                if last:
                    z = (T - 2) - s2
                    nc.gpsimd.memset(a1[:, :, z:W2], 0.0)

                # ---- conv1 -> c1 dual-row fp8 (+bias1)
                c1 = c1p.tile([128, NB, 2, W3], F8, tag="c1", name="c1")

                def c1_out(o, c0, n, ps):
                    nc.scalar.activation(c1[:, o, 0, c0:c0 + n], ps[:, :n],
                                         AF.Identity, bias=sc_t[:, o, 56:57],
                                         scale=1.0 / WS)
                    if c0 == 0:
                        nc.vector.tensor_scalar(c1[:, o, 1, 0:n - 1], ps[:, 1:n],
                                                1.0 / WS, sc_t[:, o, 56:57],
                                                ALU.mult, ALU.add)
                    else:
                        nc.vector.tensor_scalar(c1[:, o, 1, c0 - 1:c0 + n - 1],
                                                ps[:, 0:n],
                                                1.0 / WS, sc_t[:, o, 56:57],
                                                ALU.mult, ALU.add)

                dense(w18, a1, W3, 56, c1_out)
                if first:
                    nc.gpsimd.memset(c1[:, :, 0, 0:0 - s3], 0.0)
                    if 0 - s3 - 1 > 0:
                        nc.gpsimd.memset(c1[:, :, 1, 0:0 - s3 - 1], 0.0)
                if last:
                    z = (T - 2) - s3
                    nc.gpsimd.memset(c1[:, :, 0, z:W3], 0.0)
                    nc.gpsimd.memset(c1[:, :, 1, z - 1:W3], 0.0)

                # ---- stage 2: up+snake -> down -> a2 (fp8)
                a2 = amidp.tile([128, NB, W5], F8, tag="amid", name="a2")
                Ss = []
                for b in range(NB):
                    S = stp.tile([128, 2, W4], F8, tag=f"S{b}", name="S2")
                    upsnake(b, c1, W4, 1, S)
                    if first:
                        nc.gpsimd.memset(S[:, 0, 0:-s4], 0.0)
                        nc.gpsimd.memset(S[:, 1, 0:1 - s4], 0.0)
                    if last:
                        z = (T - 3) - s4
                        nc.gpsimd.memset(S[:, 0, z - 1:W4], 0.0)
                        nc.gpsimd.memset(S[:, 1, z:W4], 0.0)
                    Ss.append(S)
                for b in range(NB):
                    downconv(b, Ss[b], W5, 1, a2)
                if first:
                    nc.gpsimd.memset(a2[:, :, 0:0 - s5], 0.0)
                if last:
                    z = (T - 4) - s5
                    nc.gpsimd.memset(a2[:, :, z:W5], 0.0)

                # ---- conv2 (+bias2) + residual -> out
                outt = outp.tile([128, NB, Lo], BF16, tag="outt", name="outt")

                def c2_out(o, c0, n, ps):
                    nc.scalar.activation(outt[:, o, c0:c0 + n], ps[:, :n],
                                         AF.Identity, bias=sc_t[:, o, 57:58],
                                         scale=1.0 / WS)

                dense(w28, a2, Lo, 57, c2_out, residual_src=xbt)
                nc.sync.dma_start(out_d[:, :, t0:t0 + Lo], outt[:, :, 0:Lo])
    nc.finalize()
    return nc


def _prep_host(x, up_w1, down_w1, alpha1, beta1, up_w2, down_w2, alpha2, beta2,
               c1_w, c1_b, c2_w, c2_b):
    bf = ml_dtypes.bfloat16
    f8 = ml_dtypes.float8_e4m3
    B = x.shape[0]
    ar = np.arange(128)

    def dense_w8(w):
        # [128ci_local, 3k, 2P, 2i, NBo, 128co] ; ci = (2P+i)*128 + ci_local
        out = np.empty((128, 3, 2, 2, NB, 128), np.float32)
        wr = w.reshape(NB, 128, NB, 128, 3)  # o, co, i, ci, k
        for P in range(2):
            for ii in range(2):
                # in-block index 2P+ii -> (ci_local, k, o, co)
                out[:, :, P, ii, :, :] = wr[:, :, 2 * P + ii, :, :].transpose(2, 3, 0, 1)
        return (out * WS).astype(f8)

    w18 = dense_w8(c1_w)
    w28 = dense_w8(c2_w)

    up8 = np.zeros((128, 2 * NB * 2 * 3, 2, 128), np.float32)
    dn8 = np.zeros((128, 2 * NB * 6, 2, 128), np.float32)
    sc = np.zeros((128, NB, 64), np.float32)
    cidx = np.arange(C)
    for s, (up_w, down_w, alpha, beta) in enumerate(
        ((up_w1, down_w1, alpha1, beta1), (up_w2, down_w2, alpha2, beta2))
    ):
        off = s * S2
        a2v = 2.0 * np.exp(alpha)
        inv2b = 1.0 / (2.0 * np.exp(beta) + 1e-9)
        for b in range(NB):
            cs = cidx[b * 128:(b + 1) * 128]
            for phase in range(2):
                for p in range(3):
                    si = ((s * NB + b) * 2 + phase) * 3 + p
                    up8[ar, si, 0, ar] = up_w[2 * cs + phase, 2 * p] * US
                    up8[ar, si, 1, ar] = up_w[2 * cs + phase, 2 * p + 1] * US
            for r in range(6):
                di = (s * NB + b) * 6 + r
                dn8[ar, di, 0, ar] = down_w[cs, 2 * r + 1] * DS  # d_e[r]
                dn8[ar, di, 1, ar] = down_w[cs, 2 * r] * DS      # d_o[r]
            sc[:, b, off + 24] = a2v[cs] / US
            sc[:, b, off + 25] = np.pi / 2
            sc[:, b, off + 26] = -inv2b[cs]
            sc[:, b, off + 27] = inv2b[cs]
    for b in range(NB):
        cs = cidx[b * 128:(b + 1) * 128]
        sc[:, b, 56] = c1_b[cs]
        sc[:, b, 57] = c2_b[cs]

    idr = np.zeros((128, 128), np.float32)
    idr[ar, ar] = WS
    idr = idr.astype(bf)

    up8 = up8.astype(f8)
    dn8 = dn8.astype(f8)

    in_maps = []
    for bi in range(B):
        xpad = np.zeros((C, T + 2 * PAD), np.float32)
        xpad[:, PAD:PAD + T] = x[bi]
        xr = np.ascontiguousarray(
            xpad.reshape(NB, 128, T + 2 * PAD).transpose(1, 0, 2))
        in_maps.append({
            "x8": xr.astype(f8), "xb": xr.astype(bf),
            "up8": up8, "dn8": dn8, "w18": w18, "w28": w28,
            "idr": idr, "sc": sc.astype(np.float32),
        })
    return in_maps


_NC_CACHE = None


def _install_profile_hook():
    import types

    try:
        from antenv.axon_hooks import get_axon_ntff_profile_hook  # noqa: F401
        return
    except ImportError:
        pass
    try:
        import antenv
        mod = types.ModuleType("antenv.axon_hooks")
        _state = {"hook": None}
        mod.set_axon_ntff_profile_hook = lambda h: _state.__setitem__("hook", h)
        mod.get_axon_ntff_profile_hook = lambda: _state["hook"]
        sys.modules["antenv.axon_hooks"] = mod
        antenv.axon_hooks = mod
        if "/root/.axon_site" not in sys.path:
            sys.path.insert(0, "/root/.axon_site")
        from trn_agent_boot.trn_boot import _ntff_profile_via_ctypes
        mod.set_axon_ntff_profile_hook(
            _ntff_profile_via_ctypes("/opt/axon/libaxon_pjrt.so"))
    except Exception as e:
        print(f"profile hook install failed: {e}")


def kernel(**inputs):
    global _NC_CACHE, LAST_EXEC_NS, LAST_PROFILE
    import os

    args = {k: np.asarray(v) for k, v in inputs.items()}
    in_maps = _prep_host(**args)
    if _NC_CACHE is None:
        _NC_CACHE = build_graph()
    nc = _NC_CACHE
    trace = bool(os.environ.get("KERNEL_TRACE"))
    kw = {}
    if trace:
        _install_profile_hook()
        kw["tmpdir"] = os.environ.get("KERNEL_TRACE_DIR", "/tmp/ktrace")
        os.makedirs(kw["tmpdir"], exist_ok=True)
    res = run_bass_kernel_spmd(
        nc, in_maps, core_ids=list(range(8)), trace=trace, **kw,
    )
    LAST_EXEC_NS = res.exec_time_ns
    LAST_PROFILE = res.profile_json
    B = len(in_maps)
    out = np.empty((B, C, TOUT), np.float32)
    for bi in range(B):
        o = np.asarray(res.results[bi]["out"]).astype(np.float32)
        out[bi] = o.transpose(1, 0, 2).reshape(C, TOUT)
    return out
